# revision 38
# baseline (speedup 1.0000x reference)
"""DeepSet encoder (phi MLP -> sum/max pool -> rho MLP) as a Trainium2 Bass kernel.

Sharding: data-parallel over the batch dim. 64 samples -> 8 cores x 8 samples.
Weights are replicated on every core; no cross-core communication.

On-chip layout is feature-major: activations live as [feature_partition,
set_free] tiles so matmul contraction is on the partition dim, the bias is a
per-partition scalar, and pooling over the set dim is a free-axis reduction.

Both phi matmuls run in fp8e4m3 with DoubleRow (2 fp8 rows per PE cell ->
256-row contraction per pass, 2x bf16 throughput). W2 is quantized with
per-column error feedback so the quantization error stays orthogonal to the
large mean component of the h1 sums (else it dominates the sum-pool). h1 is
written directly in fp8 by the phi1 epilogues, split across ScalarE and
VectorE; the sum pool rides ScalarE's activation accumulator, the max pool is
a VectorE reduce over h2. rho stays fp16, with its 16 per-tile accumulators
packed into per-engine PSUM banks (one zero-region start per bank) and its
k-bursts interleaved into phi2 of the last sample so they chase the epilogue
cast chain. Epilogue/pool destinations are separate small tiles per feature
tile because the dependency tracker is tile-granular.

Self-contained: only relies on the system-installed concourse/bass stack.
"""

import sys

import numpy as np

for _p in ("/opt/trn_rl_repo",):
    if _p not in sys.path:
        sys.path.insert(0, _p)

import ml_dtypes  # noqa: E402

import concourse.bass as bass  # noqa: E402,F401
import concourse.mybir as mybir  # noqa: E402
import concourse.tile as tile  # noqa: E402
from concourse import bacc  # noqa: E402
from concourse.bass_utils import run_bass_kernel_spmd  # noqa: E402

FP16 = mybir.dt.float16
FP32 = mybir.dt.float32
NP_FP16 = np.float16
# fp8e4m3: x ~ N(0,1), W1 ~ U(+-0.044), W2 ~ U(+-0.031), h1 in [0, ~4] all sit
# inside TRN e4m3's +-240 range (subnormals are honored, so W2's sub-2^-6 half
# keeps its 2^-9 quantization step).
FP8 = mybir.dt.float8e4
NP_FP8 = ml_dtypes.float8_e4m3
DOUBLE_ROW = mybir.MatmulPerfMode.DoubleRow

B, N, D_IN, D_H = 64, 512, 512, 1024
N_CORES = 8
BL = B // N_CORES  # samples per core
NPAIR = BL // 2  # xt DMA granularity: sample pairs
P = 128
K2 = D_H // P  # feature tiles of D_H (8)
KK1 = D_IN // 256  # phi1 DoubleRow chunks (2)
KK2 = D_H // 256  # phi2 DoubleRow chunks (4)
KR1 = 2 * D_H // P  # rho1 contraction tiles (16)
N_WARM = 24  # PE warm-up matmuls (cover DMA startup latency + clock ramp)

RELU = mybir.ActivationFunctionType.Relu
OP_ADD = mybir.AluOpType.add
OP_MAX = mybir.AluOpType.max
NEG_BIG = -3.0e38


def build_program() -> bacc.Bacc:
    nc = bacc.Bacc("TRN2", target_bir_lowering=False, debug=False, num_devices=N_CORES)

    # All staged host-side into the exact SBUF tile layouts so every DMA is
    # contiguous per partition:
    #   xt[g, p, s, kk, j, n] = x[g*2+s, n, kk*256 + j*128 + p]     (fp8)
    #   w1[p, kk, j, h] = W1[kk*256 + j*128 + p, h]                 (fp8)
    #   w2[p, kk, j, h] = W2[kk*256 + j*128 + p, h]                 (fp8)
    #   wr[p, k, h] = Wr1[k*128 + p, h] (k<16) / Wr2[(k-16)*128+p, h]  (fp16)
    #   bias[p, i, m] = b_i[m*128 + p]  (i = b1, b2, br1, br2)      (fp32)
    xt_d = nc.dram_tensor("xt", [NPAIR, P, 2, KK1, 2, N], FP8, kind="ExternalInput").ap()
    w1_d = nc.dram_tensor("w1", [P, KK1, 2, D_H], FP8, kind="ExternalInput").ap()
    w2_d = nc.dram_tensor("w2", [P, KK2, 2, D_H], FP8, kind="ExternalInput").ap()
    wr_d = nc.dram_tensor("wr", [P, KR1 + K2, D_H], FP16, kind="ExternalInput").ap()
    bias_d = nc.dram_tensor("bias", [P, 4, K2], FP32, kind="ExternalInput").ap()
    # out[p, m, s] = r2[m*128 + p, s]  (feature-major, host transposes back)
    out_d = nc.dram_tensor("out", [P, K2, BL], FP32, kind="ExternalOutput").ap()

    with tile.TileContext(nc) as tc:
        with (
            tc.tile_pool(name="const", bufs=1) as cpool,
            tc.tile_pool(name="h1", bufs=2) as h1pool,
            tc.tile_pool(name="h2", bufs=2) as h2pool,
            tc.tile_pool(name="jk", bufs=4) as jkpool,
            tc.tile_pool(name="ps", bufs=8, space="PSUM") as pspool,
        ):
            # --- PE warm-up ---
            # The PE clock needs ~3us of sustained activity to reach 2.4GHz.
            # Burn the DMA-startup window on dummy matmuls over a zeroed tile;
            # phi1(0) then rides the tail of the ramp.
            warm_sb = cpool.tile([P, N], FP16)
            nc.gpsimd.memset(warm_sb[:], 0.0)
            for i in range(N_WARM):
                wps = pspool.tile([P, N], FP32, tag="ps", name=f"warm{i}")
                nc.tensor.matmul(wps[:], warm_sb[:, 0:P], warm_sb[:], start=True, stop=True)

            # --- persistent SBUF state ---
            # The sync sequencer issues one DIRECT2D per ~0.6us, so issue
            # order = time order. First-needed bytes first: w1/xt0 kk-chunks
            # interleaved so phi1(0)'s kk=0 passes can start after ~512KB.
            w1_sb = cpool.tile([P, KK1, 2, D_H], FP8)
            xt_sb = [cpool.tile([P, 2, KK1, 2, N], FP8, name=f"xt{g}") for g in range(NPAIR)]
            for kk in range(KK1):
                nc.sync.dma_start(w1_sb[:, kk], w1_d[:, kk])
                nc.sync.dma_start(xt_sb[0][:, :, kk], xt_d[0, :, :, kk])
            bias_sb = cpool.tile([P, 4, K2], FP32)
            nc.sync.dma_start(bias_sb[:], bias_d)
            w2_sb = cpool.tile([P, KK2, 2, D_H], FP8)
            nc.sync.dma_start(w2_sb[:], w2_d)
            for g in range(1, NPAIR):
                nc.sync.dma_start(xt_sb[g][:], xt_d[g])
            wr_sb = cpool.tile([P, KR1 + K2, D_H], FP16)
            nc.sync.dma_start(wr_sb[:], wr_d)

            # pooled vectors as per-feature-tile tensors: the dependency
            # tracker is tile-granular, so a shared tile would make every
            # rho1 matmul wait on ALL pool writes instead of just its own k.
            # k < K2: sum half; k >= K2: max half.
            pooled = [cpool.tile([P, BL], FP32, name=f"pool_{k}") for k in range(KR1)]
            pooled_bf = [cpool.tile([P, BL], FP16, name=f"poolbf_{k}") for k in range(KR1)]
            r1_sb = [cpool.tile([P, BL], FP16, name=f"r1_{m}") for m in range(K2)]
            # out halves split per engine (ScalarE: a, VectorE: b) so the two
            # epilogue chains never write the same tile.
            outa_sb = cpool.tile([P, K2 // 2, BL], FP32)
            outb_sb = cpool.tile([P, K2 // 2, BL], FP32)

            def h1_tile(b):
                # h1 in fp8 DoubleRow pairing for phi2: [P, kk2, j, N] with
                # feature f = kk2*256 + j*128 + p; the phi1 m-tile (m*128+p)
                # lands at (kk2, j) = (m//2, m%2).
                return h1pool.tile([P, KK2, 2, N], FP8, tag="h1", name=f"h1_{b}")

            def phi1_ep(b, h1_sb, m, ps):
                # relu(psum + b1) -> fp8; 3 of 8 on ScalarE, rest VectorE
                dst = h1_sb[:, m // 2, m % 2, :]
                if m % 3 == 0:
                    nc.scalar.activation(
                        dst, ps[:], RELU, bias=bias_sb[:, 0, m : m + 1], scale=1.0
                    )
                else:
                    nc.vector.tensor_scalar(
                        dst, ps[:], bias_sb[:, 0, m : m + 1], 0.0, OP_ADD, OP_MAX
                    )

            def phi1_m(b, h1_sb, m):
                xt = xt_sb[b // 2][:, b % 2]
                ps = pspool.tile([P, N], FP32, tag="ps", name=f"ps1_{b}_{m}")
                for kk in range(KK1):
                    nc.tensor.matmul(
                        ps[:], w1_sb[:, kk, :, m * P : (m + 1) * P], xt[:, kk],
                        perf_mode=DOUBLE_ROW, start=(kk == 0), stop=(kk == KK1 - 1),
                    )
                phi1_ep(b, h1_sb, m, ps)

            def phi1_0(h1_sb):
                # sample 0, kk-major so the first 8 passes only need the kk=0
                # chunks of the w1/xt DMAs
                xt = xt_sb[0][:, 0]
                ps1 = []
                for m in range(K2):
                    ps = pspool.tile([P, N], FP32, tag="ps", name=f"ps1_0_{m}")
                    ps1.append(ps)
                    nc.tensor.matmul(
                        ps[:], w1_sb[:, 0, :, m * P : (m + 1) * P], xt[:, 0],
                        perf_mode=DOUBLE_ROW, start=True, stop=False,
                    )
                for m in range(K2):
                    ps = ps1[m]
                    nc.tensor.matmul(
                        ps[:], w1_sb[:, 1, :, m * P : (m + 1) * P], xt[:, 1],
                        perf_mode=DOUBLE_ROW, start=False, stop=True,
                    )
                    phi1_ep(0, h1_sb, m, ps)

            def phi2_m(b, h1_sb, m):
                ps = pspool.tile([P, N], FP32, tag="ps", name=f"ps2_{b}_{m}")
                for kk in range(KK2):
                    nc.tensor.matmul(
                        ps[:], w2_sb[:, kk, :, m * P : (m + 1) * P], h1_sb[:, kk],
                        perf_mode=DOUBLE_ROW, start=(kk == 0), stop=(kk == KK2 - 1),
                    )
                # sum pool: relu(psum + b2) through ScalarE with the
                # activation accumulator; the written h2 tile is scratch.
                h2_sb = h2pool.tile([P, N], FP16, tag="h2", name=f"h2_{b}_{m}")
                nc.scalar.activation(
                    h2_sb[:], ps[:], RELU,
                    bias=bias_sb[:, 1, m : m + 1], scale=1.0,
                    accum_out=pooled[m][:, b : b + 1],
                )
                # max pool over the relu'd h2 on VectorE.
                nc.vector.tensor_reduce(
                    pooled[K2 + m][:, b : b + 1], h2_sb[:],
                    axis=mybir.AxisListType.X, op=OP_MAX,
                )
                if b == BL - 1:
                    # final sample: finish the pooled vectors per tile so
                    # rho1's matmul bursts can chase the epilogue chain.
                    nc.vector.tensor_copy(pooled_bf[m][:], pooled[m][:])
                    nc.vector.tensor_copy(pooled_bf[K2 + m][:], pooled[K2 + m][:])

            # software pipeline, interleaved per m-tile: phi2(b-1, m) and
            # phi1(b, m) alternate so PSUM allocations spread across the
            # stage — block-major emission made phi2(b)'s first matmuls WAR-
            # wait on phi1(b+1)'s trailing epilogues to free PSUM buffers.
            h1_tiles = [h1_tile(0)]
            phi1_0(h1_tiles[0])
            for b in range(1, BL):
                h1_tiles.append(h1_tile(b))
                if b < BL - 1:
                    for m in range(K2):
                        phi2_m(b - 1, h1_tiles[b - 1], m)
                        phi1_m(b, h1_tiles[b], m)
                else:
                    # last stage un-interleaved, phi1 first: its epilogues
                    # drain early so ScalarE enters phi2(7) with a shallow
                    # backlog — the ep(7,m) -> max -> cast chain paces the
                    # whole rho tail.
                    for m in range(K2):
                        phi1_m(b, h1_tiles[b], m)
                    for m in range(K2):
                        phi2_m(b - 1, h1_tiles[b - 1], m)

            # --- rho MLP over the 8 pooled vectors (feature-major, free=8) ---
            # All 16 rho1 m-accumulators share one PSUM bank as [P, 8] slices.
            # k-burst order: max half first (ready early via the TTR path),
            # then sum k=0..6 (chasing the ScalarE epilogue chain), then the
            # two stragglers (k=15 needs TTR(7,7), k=7 needs the last cast).
            # One PSUM bank holds all 8 m-accumulators as [P, 8] slices: the
            # first matmul's start zeroes the whole 2KB zero region, so every
            # other pass accumulates (fresh bytes overwrite pending-zero).
            # Per-engine PSUM banks (A: ScalarE's m=0..3, B: VectorE's m=4..7)
            # so the two epilogue chains never touch the same PSUM tile —
            # same-tile cross-engine access is serialized by the framework.
            rho1_psa = pspool.tile([P, N], FP32, tag="ps", name="rho1_psa")
            rho1_psb = pspool.tile([P, N], FP32, tag="ps", name="rho1_psb")
            H = K2 // 2

            def rho1_burst(k, first, last):
                for m in range(K2):
                    ps = rho1_psa if m < H else rho1_psb
                    nc.tensor.matmul(
                        ps[:, (m % H) * BL : (m % H + 1) * BL],
                        wr_sb[:, k, m * P : (m + 1) * P],
                        pooled_bf[k][:],
                        start=(first and m % H == 0),
                        stop=(last and m % H == H - 1),
                    )

            # last sample's phi2 with rho1 k-bursts interleaved: the sum/max
            # tiles for k = j-3 are cast by the time phi2(7, j)'s matmuls
            # retire (ScalarE first drains the previous stage's epilogue
            # backlog), so the bursts chase the cast chain without stalling
            # the in-order PE. Stragglers go after the loop.
            burst_sched = {j: [j - 4, K2 + j - 4] for j in range(4, K2)}
            tail_bursts = [4, 12, 5, 13, 6, 14, 7, 15]
            first_k = burst_sched[4][0]
            last_k = tail_bursts[-1]
            for j in range(K2):
                phi2_m(BL - 1, h1_tiles[BL - 1], j)
                for k in burst_sched.get(j, []):
                    rho1_burst(k, k == first_k, k == last_k)
            for k in tail_bursts:
                rho1_burst(k, k == first_k, k == last_k)
            # rho epilogues: ScalarE owns m=0..3, VectorE owns m=4..7, each
            # writing its own destination tile so the two chains share no
            # tiles and run fully in parallel. Emission interleaved so both
            # engines start immediately.
            for mm in range(H):
                for half, eng_m in ((0, mm), (1, H + mm)):
                    src = rho1_psa if half == 0 else rho1_psb
                    ps = src[:, (eng_m % H) * BL : (eng_m % H + 1) * BL]
                    dst = r1_sb[eng_m][:]
                    if half == 0:
                        nc.scalar.activation(
                            dst, ps, RELU,
                            bias=bias_sb[:, 2, eng_m : eng_m + 1], scale=1.0,
                        )
                    else:
                        nc.vector.tensor_scalar(
                            dst, ps,
                            bias_sb[:, 2, eng_m : eng_m + 1], 0.0, OP_ADD, OP_MAX,
                        )
            rho2_psa = pspool.tile([P, N], FP32, tag="ps", name="rho2_psa")
            rho2_psb = pspool.tile([P, N], FP32, tag="ps", name="rho2_psb")
            for k in range(K2):
                r1k = r1_sb[k][:]
                for m in range(K2):
                    ps = rho2_psa if m < H else rho2_psb
                    nc.tensor.matmul(
                        ps[:, (m % H) * BL : (m % H + 1) * BL],
                        wr_sb[:, KR1 + k, m * P : (m + 1) * P],
                        r1k,
                        start=(k == 0 and m % H == 0),
                        stop=(k == K2 - 1 and m % H == H - 1),
                    )
            for mm in range(H):
                for half, eng_m in ((0, mm), (1, H + mm)):
                    src = rho2_psa if half == 0 else rho2_psb
                    ps = src[:, (eng_m % H) * BL : (eng_m % H + 1) * BL]
                    dst = (outa_sb if half == 0 else outb_sb)[:, eng_m % H, :]
                    if half == 0:
                        nc.scalar.activation(
                            dst, ps, RELU,
                            bias=bias_sb[:, 3, eng_m : eng_m + 1], scale=1.0,
                        )
                    else:
                        nc.vector.tensor_scalar(
                            dst, ps,
                            bias_sb[:, 3, eng_m : eng_m + 1], 0.0, OP_ADD, OP_MAX,
                        )
            nc.sync.dma_start(out_d[:, :H], outa_sb[:])
            nc.sync.dma_start(out_d[:, H:], outb_sb[:])

    return nc


_CACHE: dict = {}


def get_compiled() -> bacc.Bacc:
    if "nc" not in _CACHE:
        nc = build_program()
        nc.compile()
        _CACHE["nc"] = nc
    return _CACHE["nc"]


def stage_inputs(x, W_phi1, b_phi1, W_phi2, b_phi2, W_rho1, b_rho1, W_rho2, b_rho2):
    """Host-side staging: transpose x, quantize, pack weights/biases."""

    def q8_feedback(a):
        # fp8e4m3 quantization with per-column error feedback down the
        # contraction dim: keeps each column's cumulative quantization error
        # bounded by half a step, so the error stays orthogonal to the large
        # mean component of the activation sums (the sum-pool path).
        a = np.asarray(a, np.float32)
        q = np.empty_like(a)
        err = np.zeros(a.shape[1], np.float32)
        for k in range(a.shape[0]):
            v = a[k] + err
            q[k] = v.astype(NP_FP8).astype(np.float32)
            err = v - q[k]
        return q.astype(NP_FP8)

    def w8(a, kk):
        # [kk*256, H] -> [P, kk, 2, H] with w[p, c, j, h] = W[c*256 + j*128 + p, h]
        a = q8_feedback(a)
        return np.ascontiguousarray(a.reshape(kk, 2, P, -1).transpose(2, 0, 1, 3))

    def w16(a):
        # [KO*P, H] -> [P, KO, H] with w[p, ko, h] = W[ko*P + p, h]
        a = np.asarray(a, np.float32).astype(NP_FP16)
        ko = a.shape[0] // P
        return a.reshape(ko, P, -1).transpose(1, 0, 2)

    def btile(a):
        # [n_tiles*P] -> [P, n_tiles] with b_sb[p, m] = b[m*P + p]
        return np.asarray(a, np.float32).reshape(-1, P).T

    # x[b, n, d] -> xt[g, p, s, kk, j, n] = x[g*2+s, n, kk*256 + j*128 + p]
    xt = np.asarray(x, np.float32).astype(NP_FP8)
    xt = xt.reshape(B // 2, 2, N, KK1, 2, P).transpose(0, 5, 1, 3, 4, 2)
    xt = np.ascontiguousarray(xt)  # [B//2, P, 2, KK1, 2, N]
    wr = np.ascontiguousarray(
        np.concatenate([w16(W_rho1), w16(W_rho2)], axis=1)
    )
    bias = np.ascontiguousarray(
        np.stack([btile(b_phi1), btile(b_phi2), btile(b_rho1), btile(b_rho2)], axis=1)
    )
    shared = {
        "w1": w8(W_phi1, KK1),
        "w2": w8(W_phi2, KK2),
        "wr": wr,
        "bias": bias,
    }
    in_maps = []
    for c in range(N_CORES):
        m = dict(shared)
        m["xt"] = np.ascontiguousarray(xt[c * NPAIR : (c + 1) * NPAIR])
        in_maps.append(m)
    return in_maps


def gather_output(results) -> np.ndarray:
    # per-core out: [P, K2, BL] with out[p, m, s] = r2[m*128+p, s]
    parts = []
    for c in range(N_CORES):
        o = np.asarray(results[c]["out"], np.float32)  # [P, K2, BL]
        parts.append(o.transpose(2, 1, 0).reshape(BL, D_H))  # [BL, D_H]
    return np.concatenate(parts, axis=0)


def run(trace: bool = False, **inputs):
    nc = get_compiled()
    in_maps = stage_inputs(**inputs)
    res = run_bass_kernel_spmd(nc, in_maps, core_ids=list(range(N_CORES)), trace=trace)
    return gather_output(res.results), res


def kernel(**inputs) -> np.ndarray:
    out, _ = run(trace=False, **inputs)
    return out


# revision 40
# speedup vs baseline: 1.0018x; 1.0018x over previous
"""DeepSet encoder (phi MLP -> sum/max pool -> rho MLP) as a Trainium2 Bass kernel.

Sharding: data-parallel over the batch dim. 64 samples -> 8 cores x 8 samples.
Weights are replicated on every core; no cross-core communication.

On-chip layout is feature-major: activations live as [feature_partition,
set_free] tiles so matmul contraction is on the partition dim, the bias is a
per-partition scalar, and pooling over the set dim is a free-axis reduction.

Both phi matmuls run in fp8e4m3 with DoubleRow (2 fp8 rows per PE cell ->
256-row contraction per pass, 2x bf16 throughput). W2 is quantized with
per-column error feedback so the quantization error stays orthogonal to the
large mean component of the h1 sums (else it dominates the sum-pool). h1 is
written directly in fp8 by the phi1 epilogues, split across ScalarE and
VectorE; the sum pool rides ScalarE's activation accumulator, the max pool is
a VectorE reduce over h2. rho stays fp16, with its 16 per-tile accumulators
packed into per-engine PSUM banks (one zero-region start per bank) and its
k-bursts interleaved into phi2 of the last sample so they chase the epilogue
cast chain. Epilogue/pool destinations are separate small tiles per feature
tile because the dependency tracker is tile-granular.

Self-contained: only relies on the system-installed concourse/bass stack.
"""

import sys

import numpy as np

for _p in ("/opt/trn_rl_repo",):
    if _p not in sys.path:
        sys.path.insert(0, _p)

import ml_dtypes  # noqa: E402

import concourse.bass as bass  # noqa: E402,F401
import concourse.mybir as mybir  # noqa: E402
import concourse.tile as tile  # noqa: E402
from concourse import bacc  # noqa: E402
from concourse.bass_utils import run_bass_kernel_spmd  # noqa: E402

FP16 = mybir.dt.float16
FP32 = mybir.dt.float32
NP_FP16 = np.float16
# fp8e4m3: x ~ N(0,1), W1 ~ U(+-0.044), W2 ~ U(+-0.031), h1 in [0, ~4] all sit
# inside TRN e4m3's +-240 range (subnormals are honored, so W2's sub-2^-6 half
# keeps its 2^-9 quantization step).
FP8 = mybir.dt.float8e4
NP_FP8 = ml_dtypes.float8_e4m3
DOUBLE_ROW = mybir.MatmulPerfMode.DoubleRow

B, N, D_IN, D_H = 64, 512, 512, 1024
N_CORES = 8
BL = B // N_CORES  # samples per core
NPAIR = BL // 2  # xt DMA granularity: sample pairs
P = 128
K2 = D_H // P  # feature tiles of D_H (8)
KK1 = D_IN // 256  # phi1 DoubleRow chunks (2)
KK2 = D_H // 256  # phi2 DoubleRow chunks (4)
KR1 = 2 * D_H // P  # rho1 contraction tiles (16)
N_WARM = 24  # PE warm-up matmuls (cover DMA startup latency + clock ramp)

RELU = mybir.ActivationFunctionType.Relu
OP_ADD = mybir.AluOpType.add
OP_MAX = mybir.AluOpType.max
NEG_BIG = -3.0e38


def build_program() -> bacc.Bacc:
    nc = bacc.Bacc("TRN2", target_bir_lowering=False, debug=False, num_devices=N_CORES)

    # All staged host-side into the exact SBUF tile layouts so every DMA is
    # contiguous per partition:
    #   xt[g, p, s, kk, j, n] = x[g*2+s, n, kk*256 + j*128 + p]     (fp8)
    #   w1[p, kk, j, h] = W1[kk*256 + j*128 + p, h]                 (fp8)
    #   w2[p, kk, j, h] = W2[kk*256 + j*128 + p, h]                 (fp8)
    #   wr[p, k, h] = Wr1[k*128 + p, h] (k<16) / Wr2[(k-16)*128+p, h]  (fp16)
    #   bias[p, i, m] = b_i[m*128 + p]  (i = b1, b2, br1, br2)      (fp32)
    xt_d = nc.dram_tensor("xt", [NPAIR, P, 2, KK1, 2, N], FP8, kind="ExternalInput").ap()
    w1_d = nc.dram_tensor("w1", [P, KK1, 2, D_H], FP8, kind="ExternalInput").ap()
    w2_d = nc.dram_tensor("w2", [P, KK2, 2, D_H], FP8, kind="ExternalInput").ap()
    wr_d = nc.dram_tensor("wr", [P, KR1 + K2, D_H], FP16, kind="ExternalInput").ap()
    bias_d = nc.dram_tensor("bias", [P, 4, K2], FP32, kind="ExternalInput").ap()
    # out[p, m, s] = r2[m*128 + p, s]  (feature-major, host transposes back)
    out_d = nc.dram_tensor("out", [P, K2, BL], FP32, kind="ExternalOutput").ap()

    with tile.TileContext(nc) as tc:
        with (
            tc.tile_pool(name="const", bufs=1) as cpool,
            tc.tile_pool(name="h1", bufs=2) as h1pool,
            tc.tile_pool(name="h2", bufs=2) as h2pool,
            tc.tile_pool(name="jk", bufs=4) as jkpool,
            tc.tile_pool(name="ps", bufs=8, space="PSUM") as pspool,
        ):
            # --- PE warm-up ---
            # The PE clock needs ~3us of sustained activity to reach 2.4GHz.
            # Burn the DMA-startup window on dummy matmuls over a zeroed tile;
            # phi1(0) then rides the tail of the ramp.
            warm_sb = cpool.tile([P, N], FP16)
            nc.gpsimd.memset(warm_sb[:], 0.0)
            for i in range(N_WARM):
                wps = pspool.tile([P, N], FP32, tag="ps", name=f"warm{i}")
                nc.tensor.matmul(wps[:], warm_sb[:, 0:P], warm_sb[:], start=True, stop=True)

            # --- persistent SBUF state ---
            # The sync sequencer issues one DIRECT2D per ~0.6us, so issue
            # order = time order. First-needed bytes first: w1/xt0 kk-chunks
            # interleaved so phi1(0)'s kk=0 passes can start after ~512KB.
            w1_sb = cpool.tile([P, KK1, 2, D_H], FP8)
            xt_sb = [cpool.tile([P, 2, KK1, 2, N], FP8, name=f"xt{g}") for g in range(NPAIR)]
            for kk in range(KK1):
                nc.sync.dma_start(w1_sb[:, kk], w1_d[:, kk])
                nc.sync.dma_start(xt_sb[0][:, :, kk], xt_d[0, :, :, kk])
            bias_sb = cpool.tile([P, 4, K2], FP32)
            nc.sync.dma_start(bias_sb[:], bias_d)
            w2_sb = cpool.tile([P, KK2, 2, D_H], FP8)
            nc.sync.dma_start(w2_sb[:], w2_d)
            for g in range(1, NPAIR):
                nc.sync.dma_start(xt_sb[g][:], xt_d[g])
            wr_sb = cpool.tile([P, KR1 + K2, D_H], FP16)
            nc.sync.dma_start(wr_sb[:], wr_d)

            # pooled vectors as per-feature-tile tensors: the dependency
            # tracker is tile-granular, so a shared tile would make every
            # rho1 matmul wait on ALL pool writes instead of just its own k.
            # k < K2: sum half; k >= K2: max half.
            pooled = [cpool.tile([P, BL], FP32, name=f"pool_{k}") for k in range(K2)]
            pooled_bf = [cpool.tile([P, BL], FP16, name=f"poolbf_{k}") for k in range(KR1)]
            r1_sb = [cpool.tile([P, BL], FP16, name=f"r1_{m}") for m in range(K2)]
            # out halves split per engine (ScalarE: a, VectorE: b) so the two
            # epilogue chains never write the same tile.
            outa_sb = cpool.tile([P, K2 // 2, BL], FP32)
            outb_sb = cpool.tile([P, K2 // 2, BL], FP32)

            def h1_tile(b):
                # h1 in fp8 DoubleRow pairing for phi2: [P, kk2, j, N] with
                # feature f = kk2*256 + j*128 + p; the phi1 m-tile (m*128+p)
                # lands at (kk2, j) = (m//2, m%2).
                return h1pool.tile([P, KK2, 2, N], FP8, tag="h1", name=f"h1_{b}")

            def phi1_ep(b, h1_sb, m, ps):
                # relu(psum + b1) -> fp8; 3 of 8 on ScalarE, rest VectorE
                dst = h1_sb[:, m // 2, m % 2, :]
                if m % 3 == 0:
                    nc.scalar.activation(
                        dst, ps[:], RELU, bias=bias_sb[:, 0, m : m + 1], scale=1.0
                    )
                else:
                    nc.vector.tensor_scalar(
                        dst, ps[:], bias_sb[:, 0, m : m + 1], 0.0, OP_ADD, OP_MAX
                    )

            def phi1_m(b, h1_sb, m):
                xt = xt_sb[b // 2][:, b % 2]
                ps = pspool.tile([P, N], FP32, tag="ps", name=f"ps1_{b}_{m}")
                for kk in range(KK1):
                    nc.tensor.matmul(
                        ps[:], w1_sb[:, kk, :, m * P : (m + 1) * P], xt[:, kk],
                        perf_mode=DOUBLE_ROW, start=(kk == 0), stop=(kk == KK1 - 1),
                    )
                phi1_ep(b, h1_sb, m, ps)

            def phi1_0(h1_sb):
                # sample 0, kk-major so the first 8 passes only need the kk=0
                # chunks of the w1/xt DMAs
                xt = xt_sb[0][:, 0]
                ps1 = []
                for m in range(K2):
                    ps = pspool.tile([P, N], FP32, tag="ps", name=f"ps1_0_{m}")
                    ps1.append(ps)
                    nc.tensor.matmul(
                        ps[:], w1_sb[:, 0, :, m * P : (m + 1) * P], xt[:, 0],
                        perf_mode=DOUBLE_ROW, start=True, stop=False,
                    )
                for m in range(K2):
                    ps = ps1[m]
                    nc.tensor.matmul(
                        ps[:], w1_sb[:, 1, :, m * P : (m + 1) * P], xt[:, 1],
                        perf_mode=DOUBLE_ROW, start=False, stop=True,
                    )
                    phi1_ep(0, h1_sb, m, ps)

            def phi2_m(b, h1_sb, m):
                ps = pspool.tile([P, N], FP32, tag="ps", name=f"ps2_{b}_{m}")
                for kk in range(KK2):
                    nc.tensor.matmul(
                        ps[:], w2_sb[:, kk, :, m * P : (m + 1) * P], h1_sb[:, kk],
                        perf_mode=DOUBLE_ROW, start=(kk == 0), stop=(kk == KK2 - 1),
                    )
                # sum pool: relu(psum + b2) through ScalarE with the
                # activation accumulator; the written h2 tile is scratch.
                h2_sb = h2pool.tile([P, N], FP16, tag="h2", name=f"h2_{b}_{m}")
                nc.scalar.activation(
                    h2_sb[:], ps[:], RELU,
                    bias=bias_sb[:, 1, m : m + 1], scale=1.0,
                    accum_out=pooled[m][:, b : b + 1],
                )
                # max pool over the relu'd h2 on VectorE, written straight to
                # fp16 (max-reduce has no low-precision restriction): rho1's
                # max-half tiles need no separate cast.
                nc.vector.tensor_reduce(
                    pooled_bf[K2 + m][:, b : b + 1], h2_sb[:],
                    axis=mybir.AxisListType.X, op=OP_MAX,
                )
                if b == BL - 1:
                    # final sample: cast the completed sum tile so rho1's
                    # sum-half bursts can chase the epilogue chain.
                    nc.vector.tensor_copy(pooled_bf[m][:], pooled[m][:])

            # software pipeline, interleaved per m-tile: phi2(b-1, m) and
            # phi1(b, m) alternate so PSUM allocations spread across the
            # stage — block-major emission made phi2(b)'s first matmuls WAR-
            # wait on phi1(b+1)'s trailing epilogues to free PSUM buffers.
            h1_tiles = [h1_tile(0)]
            phi1_0(h1_tiles[0])
            for b in range(1, BL):
                h1_tiles.append(h1_tile(b))
                if b < BL - 1:
                    for m in range(K2):
                        phi2_m(b - 1, h1_tiles[b - 1], m)
                        phi1_m(b, h1_tiles[b], m)
                else:
                    # last stage un-interleaved, phi1 first: its epilogues
                    # drain early so ScalarE enters phi2(7) with a shallow
                    # backlog — the ep(7,m) -> max -> cast chain paces the
                    # whole rho tail.
                    for m in range(K2):
                        phi1_m(b, h1_tiles[b], m)
                    for m in range(K2):
                        phi2_m(b - 1, h1_tiles[b - 1], m)

            # --- rho MLP over the 8 pooled vectors (feature-major, free=8) ---
            # All 16 rho1 m-accumulators share one PSUM bank as [P, 8] slices.
            # k-burst order: max half first (ready early via the TTR path),
            # then sum k=0..6 (chasing the ScalarE epilogue chain), then the
            # two stragglers (k=15 needs TTR(7,7), k=7 needs the last cast).
            # One PSUM bank holds all 8 m-accumulators as [P, 8] slices: the
            # first matmul's start zeroes the whole 2KB zero region, so every
            # other pass accumulates (fresh bytes overwrite pending-zero).
            # Per-engine PSUM banks (A: ScalarE's m=0..3, B: VectorE's m=4..7)
            # so the two epilogue chains never touch the same PSUM tile —
            # same-tile cross-engine access is serialized by the framework.
            rho1_psa = pspool.tile([P, N], FP32, tag="ps", name="rho1_psa")
            rho1_psb = pspool.tile([P, N], FP32, tag="ps", name="rho1_psb")
            H = K2 // 2

            def rho1_burst(k, first, last):
                for m in range(K2):
                    ps = rho1_psa if m < H else rho1_psb
                    nc.tensor.matmul(
                        ps[:, (m % H) * BL : (m % H + 1) * BL],
                        wr_sb[:, k, m * P : (m + 1) * P],
                        pooled_bf[k][:],
                        start=(first and m % H == 0),
                        stop=(last and m % H == H - 1),
                    )

            # last sample's phi2 with rho1 k-bursts interleaved: the sum/max
            # tiles for k = j-3 are cast by the time phi2(7, j)'s matmuls
            # retire (ScalarE first drains the previous stage's epilogue
            # backlog), so the bursts chase the cast chain without stalling
            # the in-order PE. Stragglers go after the loop.
            burst_sched = {j: [j - 4, K2 + j - 4] for j in range(4, K2)}
            tail_bursts = [4, 12, 5, 13, 6, 14, 7, 15]
            first_k = burst_sched[4][0]
            last_k = tail_bursts[-1]
            for j in range(K2):
                phi2_m(BL - 1, h1_tiles[BL - 1], j)
                for k in burst_sched.get(j, []):
                    rho1_burst(k, k == first_k, k == last_k)
            for k in tail_bursts:
                rho1_burst(k, k == first_k, k == last_k)
            # rho epilogues: ScalarE owns m=0..3, VectorE owns m=4..7, each
            # writing its own destination tile so the two chains share no
            # tiles and run fully in parallel. Emission interleaved so both
            # engines start immediately.
            for mm in range(H):
                for half, eng_m in ((0, mm), (1, H + mm)):
                    src = rho1_psa if half == 0 else rho1_psb
                    ps = src[:, (eng_m % H) * BL : (eng_m % H + 1) * BL]
                    dst = r1_sb[eng_m][:]
                    if half == 0:
                        nc.scalar.activation(
                            dst, ps, RELU,
                            bias=bias_sb[:, 2, eng_m : eng_m + 1], scale=1.0,
                        )
                    else:
                        nc.vector.tensor_scalar(
                            dst, ps,
                            bias_sb[:, 2, eng_m : eng_m + 1], 0.0, OP_ADD, OP_MAX,
                        )
            rho2_psa = pspool.tile([P, N], FP32, tag="ps", name="rho2_psa")
            rho2_psb = pspool.tile([P, N], FP32, tag="ps", name="rho2_psb")
            for k in range(K2):
                r1k = r1_sb[k][:]
                for m in range(K2):
                    ps = rho2_psa if m < H else rho2_psb
                    nc.tensor.matmul(
                        ps[:, (m % H) * BL : (m % H + 1) * BL],
                        wr_sb[:, KR1 + k, m * P : (m + 1) * P],
                        r1k,
                        start=(k == 0 and m % H == 0),
                        stop=(k == K2 - 1 and m % H == H - 1),
                    )
            for mm in range(H):
                for half, eng_m in ((0, mm), (1, H + mm)):
                    src = rho2_psa if half == 0 else rho2_psb
                    ps = src[:, (eng_m % H) * BL : (eng_m % H + 1) * BL]
                    dst = (outa_sb if half == 0 else outb_sb)[:, eng_m % H, :]
                    if half == 0:
                        nc.scalar.activation(
                            dst, ps, RELU,
                            bias=bias_sb[:, 3, eng_m : eng_m + 1], scale=1.0,
                        )
                    else:
                        nc.vector.tensor_scalar(
                            dst, ps,
                            bias_sb[:, 3, eng_m : eng_m + 1], 0.0, OP_ADD, OP_MAX,
                        )
            nc.sync.dma_start(out_d[:, :H], outa_sb[:])
            nc.sync.dma_start(out_d[:, H:], outb_sb[:])

    return nc


_CACHE: dict = {}


def get_compiled() -> bacc.Bacc:
    if "nc" not in _CACHE:
        nc = build_program()
        nc.compile()
        _CACHE["nc"] = nc
    return _CACHE["nc"]


def stage_inputs(x, W_phi1, b_phi1, W_phi2, b_phi2, W_rho1, b_rho1, W_rho2, b_rho2):
    """Host-side staging: transpose x, quantize, pack weights/biases."""

    def q8_feedback(a):
        # fp8e4m3 quantization with per-column error feedback down the
        # contraction dim: keeps each column's cumulative quantization error
        # bounded by half a step, so the error stays orthogonal to the large
        # mean component of the activation sums (the sum-pool path).
        a = np.asarray(a, np.float32)
        q = np.empty_like(a)
        err = np.zeros(a.shape[1], np.float32)
        for k in range(a.shape[0]):
            v = a[k] + err
            q[k] = v.astype(NP_FP8).astype(np.float32)
            err = v - q[k]
        return q.astype(NP_FP8)

    def w8(a, kk):
        # [kk*256, H] -> [P, kk, 2, H] with w[p, c, j, h] = W[c*256 + j*128 + p, h]
        a = q8_feedback(a)
        return np.ascontiguousarray(a.reshape(kk, 2, P, -1).transpose(2, 0, 1, 3))

    def w16(a):
        # [KO*P, H] -> [P, KO, H] with w[p, ko, h] = W[ko*P + p, h]
        a = np.asarray(a, np.float32).astype(NP_FP16)
        ko = a.shape[0] // P
        return a.reshape(ko, P, -1).transpose(1, 0, 2)

    def btile(a):
        # [n_tiles*P] -> [P, n_tiles] with b_sb[p, m] = b[m*P + p]
        return np.asarray(a, np.float32).reshape(-1, P).T

    # x[b, n, d] -> xt[g, p, s, kk, j, n] = x[g*2+s, n, kk*256 + j*128 + p]
    xt = np.asarray(x, np.float32).astype(NP_FP8)
    xt = xt.reshape(B // 2, 2, N, KK1, 2, P).transpose(0, 5, 1, 3, 4, 2)
    xt = np.ascontiguousarray(xt)  # [B//2, P, 2, KK1, 2, N]
    wr = np.ascontiguousarray(
        np.concatenate([w16(W_rho1), w16(W_rho2)], axis=1)
    )
    bias = np.ascontiguousarray(
        np.stack([btile(b_phi1), btile(b_phi2), btile(b_rho1), btile(b_rho2)], axis=1)
    )
    shared = {
        "w1": w8(W_phi1, KK1),
        "w2": w8(W_phi2, KK2),
        "wr": wr,
        "bias": bias,
    }
    in_maps = []
    for c in range(N_CORES):
        m = dict(shared)
        m["xt"] = np.ascontiguousarray(xt[c * NPAIR : (c + 1) * NPAIR])
        in_maps.append(m)
    return in_maps


def gather_output(results) -> np.ndarray:
    # per-core out: [P, K2, BL] with out[p, m, s] = r2[m*128+p, s]
    parts = []
    for c in range(N_CORES):
        o = np.asarray(results[c]["out"], np.float32)  # [P, K2, BL]
        parts.append(o.transpose(2, 1, 0).reshape(BL, D_H))  # [BL, D_H]
    return np.concatenate(parts, axis=0)


def run(trace: bool = False, **inputs):
    nc = get_compiled()
    in_maps = stage_inputs(**inputs)
    res = run_bass_kernel_spmd(nc, in_maps, core_ids=list(range(N_CORES)), trace=trace)
    return gather_output(res.results), res


def kernel(**inputs) -> np.ndarray:
    out, _ = run(trace=False, **inputs)
    return out
